# revision 1
# baseline (speedup 1.0000x reference)
"""Multi-head attention TRN2 Bass kernel, head-sharded across 8 NeuronCores.

Problem: S=2048, E=1024, H=16 heads, dk=dv=64, fp32.
    Q = x @ Wq.T ; K = x @ Wk.T ; V = x @ Wv.T   (per-head slices)
    A_h = softmax(Q_h K_h^T / 8) V_h
    out = concat_h(A_h) @ Wo.T
Sharding: tensor-parallel over heads. Core i owns heads (2i, 2i+1); the 8
partial [2048,1024] outputs are summed on the host.

v4 layout (per-core):
  * x arrives in 4 sequence-quarters (host pre-arranged [P, NSQB, NE, SQB]
    bf16, 2-3KB DMA lines, 3 queues per quarter), and the K/Q projections
    evacuate per-quarter through the AV PSUM pool, so block-0 scores/exp
    start as quarters land instead of after the full 4MB DMA.
  * scores: zero-padded per-head K=128 matmuls (kpad), single PE tile mode
    (row-tiled K=64 pairs measured slower: FWL/HAM interactions).
  * AV rides the ones-column trick for softmax denominators. Head B's
    V-block is padded to M=128 ([62 zeros | 2 ones | V]) so its attention
    output lands directly on partitions 64-127 - the normalized multiply
    writes a1t[64:128] in-lane and the old cross-partition gpsimd copy
    disappears from the block critical path.
  * reciprocals run straight from PSUM (no staging copy).
All matmul operands bf16 (fp32 PSUM accumulation).
"""

import numpy as np
import ml_dtypes

import concourse.mybir as mybir
import concourse.tile as tile
from concourse import bacc
from concourse.bass_utils import run_bass_kernel_spmd

S, E, H, DK, DV = 2048, 1024, 16, 64, 64
NCORES = 8
HPC = H // NCORES          # heads per core = 2
CSL = HPC * DV             # concat-dim columns per core = 128
P = 128
NE = E // P                # 8 contraction chunks for projections
SQB = 512                  # sequence block (PSUM-bank-limited matmul width)
NSQB = S // SQB            # 4
NCH = S // P               # 16 sk chunks of 128
F32 = mybir.dt.float32
BF16 = mybir.dt.bfloat16
SCALE = 1.0 / np.sqrt(DK).astype(np.float32)  # 1/8

EXP = mybir.ActivationFunctionType.Exp
MULT = mybir.AluOpType.mult

_cache = {}
last_results = None  # BassKernelResults of the most recent run (for test.py)
TRACE = False


def _build_nc():
    nc = bacc.Bacc("TRN2", target_bir_lowering=False, debug=False)

    xT = nc.dram_tensor("xT", [P, NSQB, NE, SQB], BF16, kind="ExternalInput")
    wqT = nc.dram_tensor("wqT", [P, NE, CSL], BF16, kind="ExternalInput")
    wkT = nc.dram_tensor("wkT", [P, NE, CSL], BF16, kind="ExternalInput")
    wvT = nc.dram_tensor("wvT", [P, NE, CSL], BF16, kind="ExternalInput")
    woT = nc.dram_tensor("woT", [CSL, E], BF16, kind="ExternalInput")
    ident = nc.dram_tensor("ident", [P, P], BF16, kind="ExternalInput")
    y = nc.dram_tensor("y", [S, E], BF16, kind="ExternalOutput")

    xT_r = xT.ap()
    w_r = {"q": wqT.ap(), "k": wkT.ap(), "v": wvT.ap()}
    y_ap = y.ap()

    with tile.TileContext(nc) as tc:
        with tc.tile_pool(name="persist", bufs=1) as persist, \
             tc.tile_pool(name="xw", bufs=1) as xw:
            qt = persist.tile([P, S], BF16)          # QT, both heads stacked
            kpad = [
                persist.tile([P, S], BF16, name=f"kpad{h}", tag=f"kpad{h}")
                for h in range(HPC)
            ]
            # head A V-block: [V(64) | ones(2)]; head B: [62 zeros | ones(2) | V(64)]
            vaug0 = persist.tile([P, NCH, DV + 2], BF16, name="vaug0", tag="vaug0")
            vaug1 = persist.tile([P, NCH, P], BF16, name="vaug1", tag="vaug1")
            wosb = persist.tile([P, E], BF16)
            idsb = persist.tile([P, P], BF16, name="idsb", tag="idsb")
            vt = persist.tile([P, S], BF16, name="vt", tag="vt")

            nc.gpsimd.memset(kpad[0][DK:P, :], 0.0)
            nc.gpsimd.memset(kpad[1][0:DK, :], 0.0)
            # head B denominators land on partitions 32-33 (custom-DVE reads
            # need a 32-aligned partition base)
            nc.gpsimd.memset(vaug0[:, :, DV : DV + 2], 1.0)
            nc.gpsimd.memset(vaug1[:, :, 0:32], 0.0)
            nc.gpsimd.memset(vaug1[:, :, 32:34], 1.0)
            nc.gpsimd.memset(vaug1[:, :, 34:DV], 0.0)

            # weights first (small), then x quarter-by-quarter, striped over
            # the 3 DMA-capable queues so each quarter lands as early as
            # possible and projections chase the DMA
            qs = [nc.scalar, nc.gpsimd, nc.sync]
            wsb = {}
            for i, m in enumerate(("k", "q", "v")):
                wsb[m] = xw.tile([P, NE, CSL], BF16, name=f"w{m}sb", tag=f"w{m}")
                qs[i].dma_start(wsb[m][:], w_r[m][:])
            nc.sync.dma_start(wosb[:], woT.ap())
            nc.gpsimd.dma_start(idsb[:], ident.ap())
            xq = [
                xw.tile([P, NE, SQB], BF16, name=f"xq{t}", tag=f"xq{t}")
                for t in range(NSQB)
            ]
            for t in range(NSQB):
                for p3, nsl in enumerate(
                    (slice(0, 3), slice(3, 6), slice(6, 8))
                ):
                    qs[p3].dma_start(xq[t][:, nsl, :], xT_r[:, t, nsl, :])

            # warm the ACT exp table set right after the DMA dispatches (so
            # the ~2.7us table load overlaps the input DMA without delaying
            # the scalar queue's dma_start doorbells)
            warm = persist.tile([1, 16], F32, name="warm", tag="warm")
            warm2 = persist.tile([1, 16], F32, name="warm2", tag="warm2")
            nc.gpsimd.memset(warm[:], 0.0)
            nc.scalar.activation(warm2[:], warm[:], EXP)

            # All PSUM pools coexist (4+3+1 = 8 banks); the projections
            # borrow the AV pool so evacuation is per-quarter incremental.
            with tc.tile_pool(name="sc_ps", bufs=2, space="PSUM") as sc_ps, \
                 tc.tile_pool(name="av_ps", bufs=3, space="PSUM") as av_ps, \
                 tc.tile_pool(name="op_ps", bufs=1, space="PSUM") as op_ps, \
                 tc.tile_pool(name="est", bufs=12) as est_pool, \
                 tc.tile_pool(name="a1t", bufs=2) as a1t_pool, \
                 tc.tile_pool(name="small", bufs=6) as small, \
                 tc.tile_pool(name="outp", bufs=6) as outp:

                # ---- K/Q projections, per sequence-quarter ----
                for t in range(NSQB):
                    sl = slice(t * SQB, (t + 1) * SQB)
                    pk = av_ps.tile([P, SQB], F32, name="pk", tag="av")
                    pq = av_ps.tile([P, SQB], F32, name="pq", tag="av")
                    pv = av_ps.tile([P, SQB], F32, name="pv", tag="av")
                    for n in range(NE):
                        nc.tensor.matmul(
                            pk[:], lhsT=wsb["k"][:, n, :], rhs=xq[t][:, n, :],
                            start=(n == 0), stop=(n == NE - 1),
                        )
                        nc.tensor.matmul(
                            pq[:], lhsT=wsb["q"][:, n, :], rhs=xq[t][:, n, :],
                            start=(n == 0), stop=(n == NE - 1),
                        )
                        nc.tensor.matmul(
                            pv[:], lhsT=wsb["v"][:, n, :], rhs=xq[t][:, n, :],
                            start=(n == 0), stop=(n == NE - 1),
                        )
                    nc.vector.tensor_copy(qt[:, sl], pq[:])
                    nc.vector.tensor_copy(kpad[0][0:DK, sl], pk[0:DK, :])
                    nc.vector.tensor_copy(kpad[1][DK:P, sl], pk[DK:P, :])
                    nc.vector.tensor_copy(vt[:, sl], pv[:])

                # ---- attention + output projection, per sq block ----
                # V is computed on the fly during block 0, in [sk, dv]
                # orientation: V chunk c = x[128c:128c+128] @ Wv^T.
                def emit_v_chunk(c):
                    # one fast PE transpose of the V^T slice (computed during
                    # the projection ramp) replaces 8 drain-exposed N=128
                    # matmuls per chunk
                    vp = op_ps.tile([P, P], BF16, name="vp", tag="op")
                    nc.tensor.transpose(
                        vp[:], vt[:, c * P : (c + 1) * P], idsb[:]
                    )
                    nc.vector.tensor_copy(vaug0[:, c, 0:DV], vp[:, 0:DV])
                    nc.vector.tensor_copy(vaug1[:, c, DV:P], vp[:, DV:P])

                for b in range(NSQB):
                    bsl = slice(b * SQB, (b + 1) * SQB)
                    a1t = a1t_pool.tile([P, SQB], BF16, tag="a1t")
                    at_ps = [
                        av_ps.tile([P, SQB], F32, name=f"at_ps{h}", tag="av")
                        for h in range(HPC)
                    ]
                    for g in range(NCH // 2):
                        if b == 0:
                            emit_v_chunk(2 * g)
                            emit_v_chunk(2 * g + 1)
                        pss = [
                            sc_ps.tile([P, 2 * SQB], F32, name=f"scps{h}", tag="sc")
                            for h in range(HPC)
                        ]
                        for j in range(2):
                            c = 2 * g + j
                            for h in range(HPC):
                                nc.tensor.matmul(
                                    pss[h][:, j * SQB : (j + 1) * SQB],
                                    lhsT=kpad[h][:, c * P : (c + 1) * P],
                                    rhs=qt[:, bsl],
                                    start=True, stop=True,
                                )
                        ess = []
                        for h in range(HPC):
                            es = est_pool.tile(
                                [P, 2 * SQB], BF16, name=f"est{h}", tag="est"
                            )
                            nc.scalar.activation(
                                es[:], pss[h][:], EXP, scale=float(SCALE)
                            )
                            ess.append(es)
                        for j in range(2):
                            c = 2 * g + j
                            nc.tensor.matmul(
                                at_ps[0][0 : DV + 2, :],
                                lhsT=vaug0[:, c, :],
                                rhs=ess[0][:, j * SQB : (j + 1) * SQB],
                                start=(c == 0), stop=(c == NCH - 1),
                            )
                            nc.tensor.matmul(
                                at_ps[1][:],
                                lhsT=vaug1[:, c, :],
                                rhs=ess[1][:, j * SQB : (j + 1) * SQB],
                                start=(c == 0), stop=(c == NCH - 1),
                            )
                    # normalize: A1T rows = A^T * (1/rowsum); head A lives on
                    # partitions 0-63 (denoms at 64), head B on 64-127
                    # (denoms at 62) so both multiplies stay in-lane.
                    rs0 = small.tile([1, SQB], F32, tag="rs0")
                    nc.vector.tensor_copy(rs0[:], at_ps[0][DV : DV + 1, :])
                    rsr0 = small.tile([1, SQB], F32, tag="rsr0")
                    nc.vector.reciprocal_approx_fast(rsr0[:], rs0[:])
                    bc0 = small.tile([P, SQB], F32, tag="bc0")
                    nc.gpsimd.partition_broadcast(bc0[:], rsr0[:])
                    nc.vector.tensor_tensor(
                        a1t[0:DV, :], at_ps[0][0:DV, :], bc0[0:DV, :], MULT
                    )
                    rs1 = small.tile([1, SQB], F32, tag="rs1")
                    nc.vector.tensor_copy(rs1[:], at_ps[1][32:33, :])
                    rsr1 = small.tile([1, SQB], F32, tag="rsr1")
                    nc.vector.reciprocal_approx_fast(rsr1[:], rs1[:])
                    bc1 = small.tile([P, SQB], F32, tag="bc1")
                    nc.gpsimd.partition_broadcast(bc1[:], rsr1[:])
                    nc.vector.tensor_tensor(
                        a1t[DV:P, :], at_ps[1][DV:P, :], bc1[DV:P, :], MULT
                    )

                    # output projection for this block
                    for j in range(NSQB):
                        rsl = slice(b * SQB + j * P, b * SQB + (j + 1) * P)
                        osb = outp.tile([P, E], BF16, tag="osb")
                        for e2 in range(E // SQB):
                            esl = slice(e2 * SQB, (e2 + 1) * SQB)
                            if b == NSQB - 1:
                                ops = sc_ps.tile(
                                    [P, SQB], F32, name="ops2", tag="sc"
                                )
                            else:
                                ops = op_ps.tile(
                                    [P, SQB], F32, name="ops", tag="op"
                                )
                            nc.tensor.matmul(
                                ops[:],
                                lhsT=a1t[:, j * P : (j + 1) * P],
                                rhs=wosb[:, esl],
                                start=True, stop=True,
                            )
                            if b == NSQB - 1 and e2 == 0:
                                # ScalarE is idle after the last exp; halve
                                # the tail's copy chain
                                nc.scalar.copy(osb[:, esl], ops[:])
                            else:
                                nc.vector.tensor_copy(osb[:, esl], ops[:])
                            nc.sync.dma_start(y_ap[rsl, esl], osb[:, esl])

    nc.compile()
    return nc


def kernel(x, Wq, Wk, Wv, Wo):
    global last_results
    x = np.asarray(x, dtype=np.float32)
    Wq = np.asarray(Wq, dtype=np.float32)
    Wk = np.asarray(Wk, dtype=np.float32)
    Wv = np.asarray(Wv, dtype=np.float32)
    Wo = np.asarray(Wo, dtype=np.float32)

    if "nc" not in _cache:
        _cache["nc"] = _build_nc()
    nc = _cache["nc"]

    bf = ml_dtypes.bfloat16
    # [S, E] -> [P, NSQB, NE, SQB]: xT[p, t, n, s] = x[t*SQB+s, n*P+p]
    xTq = np.ascontiguousarray(
        x.reshape(NSQB, SQB, NE, P).transpose(3, 0, 2, 1).astype(bf)
    )
    WqT = np.ascontiguousarray(Wq.T)
    WkT = np.ascontiguousarray(Wk.T)
    WvT = np.ascontiguousarray(Wv.T)
    WoT = np.ascontiguousarray(Wo.T)

    in_maps = []
    for i in range(NCORES):
        sl = slice(i * CSL, (i + 1) * CSL)

        def wslice(WT):
            # [E, CSL] slice -> [P, NE, CSL] partition-major
            return np.ascontiguousarray(
                WT[:, sl].reshape(NE, P, CSL).transpose(1, 0, 2).astype(bf)
            )

        in_maps.append({
            "xT": xTq,
            "ident": np.eye(P, dtype=np.float32).astype(bf),
            "wqT": wslice(WqT),
            "wkT": wslice(WkT),
            "wvT": wslice(WvT),
            "woT": np.ascontiguousarray(WoT[sl, :].astype(bf)),
        })

    last_results = run_bass_kernel_spmd(
        nc, in_maps, core_ids=list(range(NCORES)), trace=TRACE
    )
    out = np.zeros((S, E), dtype=np.float32)
    for r in last_results.results:
        out += r["y"].astype(np.float32)
    return out

